# revision 11
# baseline (speedup 1.0000x reference)
"""Trainium2 Bass kernel for causal self-attention (GQA + q/k RMSNorm + RoPE).

Sharding: tensor-parallel over heads across 8 NeuronCores. Core c computes
q-heads {2c, 2c+1} and their shared kv head c//2 end-to-end (projections,
attention, and the partial output projection out_c = Y_c @ wc[rows_c]); the
host sums the 8 partial outputs.

All matmuls run as float32r (TF32) with fp32 PSUM accumulation; inputs are
TF32-rounded on the host. RoPE is done in [d, token] layout with the head
dim permuted to [evens | odds] (folded into wq/wk/q_norm_w/k_norm_w host-
side), making the rotation a pair of half-plane elementwise ops.
"""

import numpy as np

B, T, C = 2, 2048, 2048
NH, NKV, HD = 16, 4, 128
NCORES = 8
HPC = NH // NCORES  # q heads per core = 2
EPS = 1e-5
ROPE_BASE = 10000.0
SCALE = 1.0 / float(np.sqrt(HD))
NEG = -100.0  # additive log-mask for causally-forbidden entries
KT = C // 128  # 16 contraction tiles for the projections
QTILE = 512
STILE = 128
NQT = T // QTILE  # 4 q-tiles per batch
NTT = T // QTILE  # token tiles per batch in projection phase

_CACHE: dict = {}


def _round_tf32(a: np.ndarray) -> np.ndarray:
    u = np.ascontiguousarray(a, dtype=np.float32).view(np.uint32).copy()
    u += 0xFFF + ((u >> 13) & 1)
    u &= np.uint32(0xFFFFE000)
    return u.view(np.float32)


def _build(reps: int = 1):
    import concourse.tile as tile
    from concourse import bacc, mybir

    F32R = mybir.dt.float32r
    F32 = mybir.dt.float32
    AF = mybir.ActivationFunctionType

    nc = bacc.Bacc("TRN2", target_bir_lowering=False, debug=False)

    def din(name, shape, dt_=F32R):
        return nc.dram_tensor(name, shape, dt_, kind="ExternalInput").ap()

    xT_d = din("xT", [C, B * T])
    wq_d = din("wq", [C, HPC * HD])
    wk_d = din("wk", [C, HD])
    wv_d = din("wv", [C, HD])
    wc_d = din("wc", [HPC * HD, C])
    cos2_d = din("cos2", [128, T], F32)
    sin2_d = din("sin2", [128, T], F32)
    lmask_d = din("lmask", [128, 4 * QTILE], F32)
    consts_d = din("consts", [128, 516])
    out_d = nc.dram_tensor("out", [B * T, C], F32, kind="ExternalOutput").ap()

    xT_re = xT_d.rearrange("(kc p) t -> p kc t", p=128)  # [128,16,4096]
    wq_re = wq_d.rearrange("(kc p) m -> p kc m", p=128)  # [128,16,256]
    wk_re = wk_d.rearrange("(kc p) m -> p kc m", p=128)  # [128,16,128]
    wv_re = wv_d.rearrange("(kc p) m -> p kc m", p=128)
    wc_re = wc_d.rearrange("(dp p) c -> p dp c", p=128)  # [128,2,2048]

    with tile.TileContext(nc) as tc:
        import contextlib

        ctx = contextlib.ExitStack()
        with ctx:
            const = ctx.enter_context(tc.tile_pool(name="const", bufs=1))
            qkv = ctx.enter_context(tc.tile_pool(name="qkv", bufs=1))
            ypool = ctx.enter_context(tc.tile_pool(name="y", bufs=1))
            xpool = ctx.enter_context(tc.tile_pool(name="x", bufs=2))
            work = ctx.enter_context(tc.tile_pool(name="wk", bufs=2))
            ptp = ctx.enter_context(tc.tile_pool(name="pt", bufs=3))
            rows = ctx.enter_context(tc.tile_pool(name="rows", bufs=2))
            outst = ctx.enter_context(tc.tile_pool(name="outst", bufs=3))
            psA = ctx.enter_context(tc.tile_pool(name="psA", bufs=4, space="PSUM"))
            psB = ctx.enter_context(tc.tile_pool(name="psB", bufs=2, space="PSUM"))
            psPV = ctx.enter_context(tc.tile_pool(name="psPV", bufs=1, space="PSUM"))
            psLS = ctx.enter_context(tc.tile_pool(name="psLS", bufs=1, space="PSUM"))

            # ---- resident weights/tables ----
            wq_sb = const.tile([128, KT, HPC * HD], F32R)
            wk_sb = const.tile([128, KT, HD], F32R)
            wv_sb = const.tile([128, KT, HD], F32R)
            wc_sb = const.tile([128, HPC, C], F32R)
            cos2 = const.tile([128, T], F32)
            sin2 = const.tile([128, T], F32)
            lmask = const.tile([128, 4 * QTILE], F32)
            consts = const.tile([128, 516], F32R)
            nc.sync.dma_start(wq_sb[:], wq_re)
            nc.sync.dma_start(wk_sb[:], wk_re)
            nc.sync.dma_start(wv_sb[:], wv_re)
            nc.sync.dma_start(wc_sb[:], wc_re)
            nc.sync.dma_start(cos2[:], cos2_d)
            nc.sync.dma_start(sin2[:], sin2_d)
            nc.sync.dma_start(lmask[:], lmask_d)
            nc.sync.dma_start(consts[:], consts_d)
            ident = consts[:, 0:128]
            ones_c = consts[:, 128:129]
            ones_r = consts[0:1, 129:257]
            qw_row = consts[0:1, 257:385]
            kw_row = consts[0:1, 385:513]
            eps_sb = consts[0:1, 513:514].bitcast(F32)

            def norm_rope_plane(ps_acc, w_row, plane_dst, b, ti):
                """RMSNorm + RoPE one [128, 512] projection tile (q or k).

                ps_acc: PSUM [128,512] raw projection (d on partitions,
                tokens on free). w_row: [1,128] norm weight (permuted).
                plane_dst: F32R SBUF [128,512] destination slice.
                """
                ts0 = ti * QTILE
                # sum of squares over d via ones-column matmul
                sq = work.tile([128, QTILE], F32R, tag="sq")
                nc.scalar.activation(sq[:], ps_acc[:], AF.Square)
                ssq = psLS.tile([1, QTILE], F32, tag="ls")
                nc.tensor.matmul(ssq[:], ones_c, sq[:], start=True, stop=True)
                # rsqrt(mean + eps) row
                rt = rows.tile([1, 2 * QTILE], F32, tag="rowsf")
                s1 = rt[0:1, 0:QTILE]
                nc.scalar.activation(s1, ssq[:], AF.Sqrt, bias=eps_sb, scale=1.0 / HD)
                r1 = rt[0:1, QTILE : 2 * QTILE]
                nc.vector.reciprocal(r1, s1)
                rtr = rows.tile([1, QTILE], F32R, tag="rowsr")
                r1r = rtr[0:1, :]
                nc.vector.tensor_copy(r1r, r1)
                # broadcast w_row (x) r1 over partitions
                bcp = psB.tile([128, QTILE], F32, tag="b")
                nc.tensor.matmul(bcp[:], w_row, r1r, start=True, stop=True)
                bc = work.tile([128, QTILE], F32, tag="bc")
                nc.scalar.copy(bc[:], bcp[:])
                # normed (fp32)
                qn = work.tile([128, QTILE], F32, tag="qn")
                nc.vector.tensor_mul(qn[:], ps_acc[:], bc[:])
                # rope (cos2/sin2 are [128, T] with the [64, T] table duplicated
                # on both partition halves so DVE base partitions match)
                cs = cos2[:, ts0 : ts0 + QTILE]
                sn = sin2[:, ts0 : ts0 + QTILE]
                t1 = work.tile([128, QTILE], F32, tag="t1")
                nc.vector.tensor_mul(t1[:], qn[:], cs)
                t2 = work.tile([128, QTILE], F32, tag="t2")
                nc.vector.tensor_mul(t2[0:64, :], qn[64:128, :], sn[64:128, :])
                nc.vector.tensor_mul(t2[64:128, :], qn[0:64, :], sn[0:64, :])
                nc.vector.tensor_sub(plane_dst[0:64, :], t1[0:64, :], t2[0:64, :])
                nc.vector.tensor_add(plane_dst[64:128, :], t1[64:128, :], t2[64:128, :])

            def body():
                for b in range(B):
                    tb = b * T
                    qT = qkv.tile([128, HPC, T], F32R, tag="qT")
                    kT = qkv.tile([128, T], F32R, tag="kT")
                    vsb = qkv.tile([128, T // 128, 128], F32R, tag="v")
                    yT = ypool.tile([128, HPC, T], F32R, tag="yT")

                    # ---- projections ----
                    for ti in range(NTT):
                        ts0 = ti * QTILE
                        acc_q0 = psA.tile([128, QTILE], F32, tag="a")
                        acc_q1 = psA.tile([128, QTILE], F32, tag="a")
                        acc_k = psA.tile([128, QTILE], F32, tag="a")
                        acc_v = psA.tile([128, QTILE], F32, tag="a")
                        for kc4 in range(KT // 4):
                            xt = xpool.tile([128, 4, QTILE], F32R, tag="xt")
                            nc.sync.dma_start(
                                xt[:],
                                xT_re[:, kc4 * 4 : kc4 * 4 + 4, tb + ts0 : tb + ts0 + QTILE],
                            )
                            for j in range(4):
                                kc = kc4 * 4 + j
                                st = kc == 0
                                sp = kc == KT - 1
                                nc.tensor.matmul(
                                    acc_q0[:], wq_sb[:, kc, 0:HD], xt[:, j, :],
                                    start=st, stop=sp,
                                )
                                nc.tensor.matmul(
                                    acc_q1[:], wq_sb[:, kc, HD : 2 * HD], xt[:, j, :],
                                    start=st, stop=sp,
                                )
                                nc.tensor.matmul(
                                    acc_k[:], wk_sb[:, kc, :], xt[:, j, :],
                                    start=st, stop=sp,
                                )
                                nc.tensor.matmul(
                                    acc_v[:], wv_sb[:, kc, :], xt[:, j, :],
                                    start=st, stop=sp,
                                )
                        # q/k: norm + rope
                        norm_rope_plane(acc_q0, qw_row, qT[:, 0, ts0 : ts0 + QTILE], b, ti)
                        norm_rope_plane(acc_q1, qw_row, qT[:, 1, ts0 : ts0 + QTILE], b, ti)
                        norm_rope_plane(acc_k, kw_row, kT[:, ts0 : ts0 + QTILE], b, ti)
                        # v: copy out then transpose to [token, d]
                        vstage = work.tile([128, QTILE], F32R, tag="vstage")
                        nc.scalar.copy(vstage[:], acc_v[:])
                        for j in range(QTILE // 128):
                            trp = psB.tile([128, 128], F32R, tag="b")
                            nc.tensor.transpose(
                                trp[:], vstage[:, j * 128 : (j + 1) * 128], ident
                            )
                            nc.scalar.copy(vsb[:, ti * 4 + j, :], trp[:].bitcast(F32))

                    # ---- attention per head ----
                    for h in range(HPC):
                        for qi in range(NQT):
                            q0 = qi * QTILE
                            n_s = 4 * qi + 4
                            ps_y = psPV.tile([128, QTILE], F32, tag="pv")
                            ps_l = psLS.tile([1, QTILE], F32, tag="ls")
                            for si in range(n_s):
                                ps_s = psB.tile([128, QTILE], F32, tag="b")
                                nc.tensor.matmul(
                                    ps_s[:],
                                    kT[:, si * 128 : (si + 1) * 128],
                                    qT[:, h, q0 : q0 + QTILE],
                                    start=True,
                                    stop=True,
                                )
                                pt = ptp.tile([128, QTILE], F32R, tag="pt")
                                j = si - 4 * qi
                                if j >= 0:
                                    sm = work.tile([128, QTILE], F32, tag="sm")
                                    nc.vector.scalar_tensor_tensor(
                                        sm[:],
                                        ps_s[:],
                                        SCALE,
                                        lmask[:, j * QTILE : (j + 1) * QTILE],
                                        op0=mybir.AluOpType.mult,
                                        op1=mybir.AluOpType.add,
                                    )
                                    nc.scalar.activation(pt[:], sm[:], AF.Exp)
                                else:
                                    nc.scalar.activation(pt[:], ps_s[:], AF.Exp, scale=SCALE)
                                st = si == 0
                                sp = si == n_s - 1
                                nc.tensor.matmul(
                                    ps_l[:], ones_c, pt[:], start=st, stop=sp
                                )
                                nc.tensor.matmul(
                                    ps_y[:], vsb[:, si, :], pt[:], start=st, stop=sp
                                )
                            # normalize: yT = ps_y * (1/l) broadcast
                            rt = rows.tile([1, 2 * QTILE], F32, tag="rowsf")
                            rl = rt[0:1, 0:QTILE]
                            nc.vector.reciprocal(rl, ps_l[:])
                            rtr = rows.tile([1, QTILE], F32R, tag="rowsr")
                            rlr = rtr[0:1, :]
                            nc.vector.tensor_copy(rlr, rl)
                            bcp = psB.tile([128, QTILE], F32, tag="b")
                            nc.tensor.matmul(bcp[:], ones_r, rlr, start=True, stop=True)
                            bc = work.tile([128, QTILE], F32, tag="ybc")
                            nc.scalar.copy(bc[:], bcp[:])
                            nc.vector.tensor_mul(
                                yT[:, h, q0 : q0 + QTILE], ps_y[:], bc[:]
                            )

                    # ---- output projection (partial over this core's heads) ----
                    for ti in range(T // 128):
                        accs = [
                            psA.tile([128, QTILE], F32, tag="a", name=f"acc_o{ci}")
                            for ci in range(4)
                        ]
                        for h in range(HPC):
                            for ci in range(4):
                                nc.tensor.matmul(
                                    accs[ci][:],
                                    yT[:, h, ti * 128 : (ti + 1) * 128],
                                    wc_sb[:, h, ci * QTILE : (ci + 1) * QTILE],
                                    start=(h == 0),
                                    stop=(h == HPC - 1),
                                )
                        for ci in range(4):
                            ob = outst.tile([128, QTILE], F32, tag="ob")
                            nc.scalar.copy(ob[:], accs[ci][:])
                            nc.sync.dma_start(
                                out_d[
                                    tb + ti * 128 : tb + (ti + 1) * 128,
                                    ci * QTILE : (ci + 1) * QTILE,
                                ],
                                ob[:],
                            )

            if reps == 1:
                body()
            else:
                with tc.For_i(0, reps, 1):
                    body()

    nc.compile()
    return nc


def _host_inputs(x, wq, wk, wv, wc, q_norm_w, k_norm_w):
    """Build the 8 per-core input dicts."""
    x2 = np.ascontiguousarray(np.asarray(x, dtype=np.float32).reshape(B * T, C))
    xT = _round_tf32(np.ascontiguousarray(x2.T))

    perm = np.concatenate([np.arange(0, HD, 2), np.arange(1, HD, 2)])
    pos = np.arange(T, dtype=np.float64)
    inv_freq = 1.0 / (ROPE_BASE ** (np.arange(0, HD, 2, dtype=np.float64) / HD))
    theta = pos[None, :] * inv_freq[:, None]  # [64, T]
    cosv = np.cos(theta).astype(np.float32)
    sinv = np.sin(theta).astype(np.float32)
    cos2 = np.concatenate([cosv, cosv], axis=0)
    sin2 = np.concatenate([sinv, sinv], axis=0)

    # lmask[p, j*512 + f] = 0 if f >= 128*j + p else NEG
    p = np.arange(128)[:, None]
    f = np.arange(QTILE)[None, :]
    lm = np.concatenate(
        [np.where(f >= 128 * j + p, 0.0, NEG).astype(np.float32) for j in range(4)],
        axis=1,
    )



    wq = np.asarray(wq, dtype=np.float32)
    wk = np.asarray(wk, dtype=np.float32)
    wv = np.asarray(wv, dtype=np.float32)
    wc = np.asarray(wc, dtype=np.float32)
    qw = np.asarray(q_norm_w, dtype=np.float32)[perm]
    kw = np.asarray(k_norm_w, dtype=np.float32)[perm]
    consts = np.zeros((128, 516), dtype=np.float32)
    consts[:, 0:128] = np.eye(128, dtype=np.float32)
    consts[:, 128] = 1.0
    consts[0, 129:257] = 1.0
    consts[0, 257:385] = _round_tf32(qw)
    consts[0, 385:513] = _round_tf32(kw)
    consts[0, 513] = EPS

    in_maps = []
    for c in range(NCORES):
        h0 = HPC * c
        g = h0 // (NH // NKV)
        qcols = np.concatenate([h * HD + perm for h in range(h0, h0 + HPC)])
        kcols = g * HD + perm
        vcols = np.arange(g * HD, (g + 1) * HD)
        wrows = np.arange(h0 * HD, (h0 + HPC) * HD)
        in_maps.append(
            {
                "xT": xT,
                "wq": _round_tf32(wq[:, qcols]),
                "wk": _round_tf32(wk[:, kcols]),
                "wv": _round_tf32(wv[:, vcols]),
                "wc": _round_tf32(wc[wrows, :]),
                "cos2": cos2,
                "sin2": sin2,
                "lmask": lm,
                "consts": consts,
            }
        )
    return in_maps


def kernel(x, wq, wk, wv, wc, q_norm_w, k_norm_w):
    from concourse.bass_utils import run_bass_kernel_spmd

    if "nc" not in _CACHE:
        _CACHE["nc"] = _build()
    nc = _CACHE["nc"]
    in_maps = _host_inputs(x, wq, wk, wv, wc, q_norm_w, k_norm_w)
    res = run_bass_kernel_spmd(nc, in_maps, core_ids=list(range(NCORES)))
    out = np.zeros((B * T, C), dtype=np.float32)
    for r in res.results:
        out += r["out"]
    return out.reshape(B, T, C)
